# revision 5
# baseline (speedup 1.0000x reference)
"""Causal multi-head attention on 8 Trainium2 NeuronCores (bf16/fp16).

Sharding: core c -> (batch b = c//2, head-group g = c%2 of 6 heads).
Host sums the two half-head partial outputs per batch.

v4 design (all-16-bit matmuls; exp as a 2-op fp16 bit trick):
  - projections: plain bf16 matmuls (K=128 x 6 k-tiles)
  - scores: bf16 K=64 matmuls; the two heads of a j-pair run in
    concurrent PE row groups (0,0)/(64,0) -> ~2x, writing one
    [128, 1024] staging tile ([head A | head B]); causal trim per tile
  - exp: 2-op Schraudolph on fp16 bits:
      pass1: y = fp16(S*k1 + c1)  (k1=32*log2e/8, c1=15*32+1024) so
             round-to-int is free in fp16's [1024,2048) window
      pass2: int16 (bits(y) - 25600) * 32 = fp16 bits of 2^t with a
             linear 5-bit mantissa; boundary tiles multiply by a
             {32|0} triangle mask instead (exact zeros)
    split between ACT (float path) and DVE (int16 2x) by booked cost
  - PV: fp16 e x fp16 V_aug [128, 65] (ones col -> denominator row 64)
  - normalize: denom rows -> spread DMA -> DVE reciprocal -> unspread ->
    K=2 sel matmul broadcasts both heads' recips -> TT mul -> zh bf16
  - W_O: plain bf16, f32 out via ACT/DVE copy + DMA
"""

import numpy as np

B = 4
S = 2048
D = 768
NH = 12
DH = 64
G = 2            # head groups (tensor parallel)
HPG = NH // G    # heads per group = 6
NP = HPG // 2    # head pairs per group = 3
ST = S // 128    # 16 s-tiles
QC = S // 512    # 4 q-chunks
N_CORES = 8
VS = 66          # per-(head,tile) stride in v_big (64 V + 1 ones + 1 pad)

FS = 32.0                                  # exponent fraction scale
K1 = FS * 1.4426950408889634 / 8.0         # pass1 scale
C1 = 15.0 * FS + 1024.0                    # pass1 bias
M32 = 32                                   # pass2 multiplier


def _split_drain_waits(nc, mybir, max_waits=1):
    """Walrus accepts one sync wait per instruction; hoist extras onto
    NoOps on the same engine (program order keeps the gating)."""
    for f in nc.m.functions:
        for bb in f.blocks:
            newlist = []
            for ins in bb.instructions:
                si = ins.sync_info
                if si is not None and si.on_wait and len(si.on_wait) > max_waits:
                    waits = list(si.on_wait)
                    for i, w in enumerate(waits[:-max_waits]):
                        d = mybir.InstNoOp(name=f"{ins.name}-sw{i}", ins=[], outs=[])
                        d.engine = ins.engine
                        d.sync_info = mybir.SyncInfo(on_wait=[w], on_update=[])
                        newlist.append(d)
                    ins.sync_info = mybir.SyncInfo(
                        on_wait=list(waits[-max_waits:]), on_update=list(si.on_update)
                    )
                newlist.append(ins)
            try:
                bb.instructions = newlist
            except Exception:
                bb.instructions.clear()
                bb.instructions.extend(newlist)


def build_program():
    import concourse.bass as bass
    import concourse.mybir as mybir
    import concourse.tile as tile
    from contextlib import ExitStack

    f32 = mybir.dt.float32
    bf16 = mybir.dt.bfloat16
    f16 = mybir.dt.float16
    i16 = mybir.dt.int16
    MULT = mybir.AluOpType.mult
    SUB = mybir.AluOpType.subtract
    COPY = mybir.ActivationFunctionType.Copy
    IDENT = mybir.ActivationFunctionType.Identity

    nc = bass.Bass("TRN2")
    xT = nc.dram_tensor("xT", [D, S], bf16, kind="ExternalInput")
    wq = nc.dram_tensor("wq", [128, 6 * 384], bf16, kind="ExternalInput")
    wk = nc.dram_tensor("wk", [128, 6 * 384], bf16, kind="ExternalInput")
    wv = nc.dram_tensor("wv", [128, 6 * 384], bf16, kind="ExternalInput")
    wo = nc.dram_tensor("wo", [128, 3 * 768], bf16, kind="ExternalInput")
    bqd = nc.dram_tensor("bqd", [128, 3], f32, kind="ExternalInput")
    bkd = nc.dram_tensor("bkd", [128, 3], f32, kind="ExternalInput")
    mskd = nc.dram_tensor("mskd", [128, 1024], i16, kind="ExternalInput")
    sel2d = nc.dram_tensor("sel2d", [2, 128], bf16, kind="ExternalInput")
    ones2_d = nc.dram_tensor("ones2_d", [128, 128], f32, kind="ExternalInput")
    out = nc.dram_tensor("out", [S, D], f32, kind="ExternalOutput")

    ew_load = {"act": 0.0, "dve": 0.0}

    def ew_pick(act_ns, dve_ns):
        ca = ew_load["act"] + act_ns + 180
        cd = ew_load["dve"] + dve_ns + 180
        if ca <= cd:
            ew_load["act"] = ca
            return "act"
        ew_load["dve"] = cd
        return "dve"

    def ew_book(which, ns):
        ew_load[which] += ns

    with tile.TileContext(nc) as tc:
        with ExitStack() as _ctx:
            _e = _ctx.enter_context
            _e(nc.allow_low_precision(reason="bf16/fp16 attention pipeline"))
            wpool = _e(tc.tile_pool(name="weights", bufs=1))
            xpool = _e(tc.tile_pool(name="xt", bufs=6))
            qkpool = _e(tc.tile_pool(name="qk", bufs=2 * NP * QC))
            vpool = _e(tc.tile_pool(name="v", bufs=1))
            ypool = _e(tc.tile_pool(name="y16", bufs=2))
            epool = _e(tc.tile_pool(name="e", bufs=3))
            zhpool = _e(tc.tile_pool(name="zh", bufs=2))
            dnpool = _e(tc.tile_pool(name="dn", bufs=2))
            rrpool = _e(tc.tile_pool(name="rr", bufs=2))
            opool = _e(tc.tile_pool(name="osb", bufs=3))
            stpool = _e(tc.tile_pool(name="st", bufs=2, space="PSUM"))
            pzpool = _e(tc.tile_pool(name="pz", bufs=3, space="PSUM"))
            mspool = _e(tc.tile_pool(name="ms", bufs=1, space="PSUM"))

            # ---- small constants ----
            sel2 = wpool.tile([2, 128], bf16, tag="sel2")
            nc.sync.dma_start(sel2[:], sel2d[:])
            bq_sb = wpool.tile([128, NP], f32, tag="bq")
            nc.sync.dma_start(bq_sb[:], bqd[:])
            bk_sb = wpool.tile([128, NP], f32, tag="bk")
            nc.sync.dma_start(bk_sb[:], bkd[:])
            msk = wpool.tile([128, 1024], i16, tag="msk")
            nc.gpsimd.dma_start(msk[:], mskd[:])
            ones2 = wpool.tile([128, 128], f32, tag="ones2")
            nc.gpsimd.dma_start(ones2[:], ones2_d[:])

            # ---- PE warmup (p-state) while input DMAs land ----
            wu = mspool.tile([128, 512], f32, tag="ms", name="wu")
            for _ in range(20):
                nc.tensor.matmul(wu[:, 0:128], ones2[:], ones2[:],
                                 start=True, stop=True)

            # ---- bulk inputs over two DMA queues ----
            xt = [xpool.tile([128, S], bf16, tag="xt", name=f"xt{a}")
                  for a in range(6)]
            for a in range(6):
                half = S // 2
                eng = nc.sync if a % 2 == 0 else nc.gpsimd
                eng.dma_start(xt[a][:, 0:half], xT[a * 128:(a + 1) * 128, 0:half])
                eng2 = nc.gpsimd if a % 2 == 0 else nc.sync
                eng2.dma_start(xt[a][:, half:S], xT[a * 128:(a + 1) * 128, half:S])
            wq_sb = wpool.tile([128, 6 * 384], bf16, tag="wq")
            nc.sync.dma_start(wq_sb[:], wq[:])
            wk_sb = wpool.tile([128, 6 * 384], bf16, tag="wk")
            nc.gpsimd.dma_start(wk_sb[:], wk[:])
            wv_sb = wpool.tile([128, 6 * 384], bf16, tag="wv")
            nc.sync.dma_start(wv_sb[:], wv[:])
            wo_sb = wpool.tile([128, 3 * 768], bf16, tag="wo")
            nc.gpsimd.dma_start(wo_sb[:], wo[:])

            # ---- persistent activation tiles ----
            qt = [[qkpool.tile([128, 512], bf16, tag="qk", name=f"qt{j}_{c}")
                   for c in range(QC)] for j in range(NP)]
            kt = [[qkpool.tile([128, 512], bf16, tag="qk", name=f"kt{j}_{c}")
                   for c in range(QC)] for j in range(NP)]
            # v_big: per (h, t): [128 kpos, VS] fp16 at offset (h*16+t)*VS
            v_big = vpool.tile([128, HPG * 16 * VS], f16, tag="v")
            vb = v_big[:].rearrange("p (h t m) -> p h t m", h=HPG, t=16)
            nc.gpsimd.memset(vb[:, :, :, 64:65], 1.0)

            zh_cs = [zhpool.tile([128, 4 * NP * 128], bf16, tag="zh",
                                 name=f"zh{c}") for c in range(QC)]

            def emit_proj_qk(c, j, which):
                w_sb, b_sb, dst = ((wq_sb, bq_sb, qt[j][c]) if which == 0
                                   else (wk_sb, bk_sb, kt[j][c]))
                ps = mspool.tile([128, 512], f32, tag="ms", name="psqk")
                for a in range(6):
                    nc.tensor.matmul(
                        ps[:], w_sb[:, a * 384 + j * 128:a * 384 + (j + 1) * 128],
                        xt[a][:, c * 512:(c + 1) * 512],
                        start=(a == 0), stop=(a == 5))
                eng = ew_pick(512 * 0.833, 512 * 1.04)
                if eng == "act":
                    nc.scalar.activation(dst[:], ps[:], IDENT,
                                         bias=b_sb[:, j:j + 1], scale=1.0)
                else:
                    nc.vector.tensor_scalar_add(dst[:], ps[:], b_sb[:, j:j + 1])

            def emit_proj_v(st):
                ps = mspool.tile([128, 512], f32, tag="ms", name="psv")
                for a in range(6):
                    nc.tensor.matmul(
                        ps[:, 0:384], xt[a][:, st * 128:(st + 1) * 128],
                        wv_sb[:, a * 384:(a + 1) * 384],
                        start=(a == 0), stop=(a == 5))
                dst = vb[:, :, st, 0:64]
                src = ps[:, 0:384].rearrange("p (h d) -> p h d", h=HPG)
                eng = ew_pick(384 * 0.833, 384 * 1.04)
                if eng == "act":
                    nc.scalar.copy(dst, src)
                else:
                    nc.vector.tensor_copy(dst, src)

            def proj_units(c):
                if c >= QC:
                    return
                for j in range(NP):
                    yield lambda j=j: emit_proj_qk(c, j, 0)
                    yield lambda j=j: emit_proj_qk(c, j, 1)
                for st in range(4 * c, 4 * c + 4):
                    yield lambda st=st: emit_proj_v(st)

            for u in proj_units(0):
                u()

            deferred = []

            for c in range(QC):
                filler = iter(proj_units(c + 1))
                zh_c = zh_cs[c]
                nt = 4 * c + 4

                for j in range(NP):
                    pz = [pzpool.tile([65, 512], f32, tag="pz",
                                      name=f"pz{c}_{j}_{hh}") for hh in range(2)]

                    def emit_scores_exp(t, c=c, j=j):
                        r = t - 4 * c
                        qoff = max(r, 0) * 128
                        w = 512 - qoff
                        kc, ko = t // 4, (t % 4) * 128
                        stg = stpool.tile([128, 1024], f32, tag="st", name="stg")
                        for hh in range(2):
                            nc.tensor.matmul(
                                stg[:, hh * 512:hh * 512 + w],
                                kt[j][kc][hh * 64:(hh + 1) * 64, ko:ko + 128],
                                qt[j][c][hh * 64:(hh + 1) * 64, qoff:qoff + w],
                                start=True, stop=True,
                            )
                        # pass1: y = fp16(S*K1 + C1), both heads in one op
                        y16 = ypool.tile([128, 1024], f16, tag="y16", name="y16")
                        ydst = y16[:].rearrange("p (hh w) -> p hh w",
                                                hh=2)[:, :, 0:w]
                        src = stg[:].rearrange("p (hh w) -> p hh w",
                                               hh=2)[:, :, 0:w]
                        eng = ew_pick(2 * w * 0.833, 2 * w * 1.04)
                        if eng == "act":
                            nc.scalar.activation(ydst, src, COPY,
                                                 bias=C1, scale=K1)
                        else:
                            nc.vector.tensor_scalar(ydst, src, K1, C1,
                                                    MULT, mybir.AluOpType.add)
                        # pass2: e bits = (bits(y) - 25600) * 32 [* mask]
                        et = epool.tile([128, 1024], i16, tag="e",
                                        name=f"e{c}_{j}_{t}")
                        edst = et[:].rearrange("p (hh w) -> p hh w",
                                               hh=2)[:, :, 0:w]
                        ysrc = y16[:].bitcast(i16).rearrange(
                            "p (hh w) -> p hh w", hh=2)[:, :, 0:w]
                        if r < 0:
                            eng = ew_pick(2 * w * 0.833, 2 * w * 0.52)
                            if eng == "act":
                                nc.scalar.activation(edst, ysrc, COPY,
                                                     bias=-25600.0 * 32.0,
                                                     scale=32.0)
                            else:
                                nc.vector.tensor_scalar(edst, ysrc, 25600,
                                                        M32, SUB, MULT)
                        else:
                            mv = msk[:].rearrange("p (hh w) -> p hh w",
                                                  hh=2)[:, :, 0:w]
                            nc.vector.scalar_tensor_tensor(edst, ysrc, 25600,
                                                           mv, SUB, MULT)
                            ew_book("dve", 2 * w * 0.52 + 180)
                        return et

                    def emit_pv(t, et, c=c, j=j, pz=pz):
                        r = t - 4 * c
                        qoff = max(r, 0) * 128
                        w = 512 - qoff
                        ev = et[:].bitcast(f16).rearrange("p (hh w) -> p hh w",
                                                          hh=2)
                        for hh in range(2):
                            h = 2 * j + hh
                            nc.tensor.matmul(
                                pz[hh][:, qoff:qoff + w],
                                vb[:, h, t, 0:65], ev[:, hh, 0:w],
                                start=(t == 0), stop=(t == nt - 1))

                    prev = [None]

                    def step(t):
                        et = emit_scores_exp(t)
                        if prev[0] is not None:
                            emit_pv(*prev[0])
                        prev[0] = (t, et)
                        if t >= 1 and deferred:
                            deferred.pop(0)()
                        else:
                            u = next(filler, None)
                            if u is not None:
                                u()

                    for t in range(nt):
                        step(t)
                    emit_pv(*prev[0])

                    # norm part A: engine-only chain (no PE instructions)
                    dn = dnpool.tile([1, 1024], f32, tag="dn", name="dn")
                    for hh in range(2):
                        eng = ew_pick(512 * 0.833, 512 * 1.04)
                        if eng == "act":
                            nc.scalar.copy(dn[:, hh * 512:(hh + 1) * 512],
                                           pz[hh][64:65, :])
                        else:
                            nc.vector.tensor_copy(dn[:, hh * 512:(hh + 1) * 512],
                                                  pz[hh][64:65, :])
                    dnp = dnpool.tile([128, 8], f32, tag="dnp", name="dnp")
                    nc.gpsimd.dma_start(dnp[:], dn[:])
                    rp = dnpool.tile([128, 8], f32, tag="rp", name="rp")
                    nc.vector.reciprocal(rp[:], dnp[:])
                    rpb = dnpool.tile([128, 8], bf16, tag="rpb", name="rpb")
                    nc.vector.tensor_copy(rpb[:], rp[:])
                    ew_book("dve", 400)
                    rr2 = rrpool.tile([2, 512], bf16, tag="rr", name="rr2")
                    nc.gpsimd.dma_start(rr2[:], rpb[:])

                    # norm part B: bcp matmul + copy + TT muls (has PE work,
                    # deferred into the next j/chunk so the PE never waits)
                    def norm_b(c=c, j=j, pz=pz, rr2=rr2, zh_c=zh_c):
                        bcp = mspool.tile([128, 512], f32, tag="ms",
                                          name="bcp")
                        nc.tensor.matmul(bcp[:], sel2[:], rr2[:],
                                         start=True, stop=True)
                        bcs = rrpool.tile([128, 512], bf16, tag="bcs",
                                          name="bcs")
                        eng = ew_pick(512 * 0.833, 512 * 1.04)
                        if eng == "act":
                            nc.scalar.copy(bcs[:], bcp[:])
                        else:
                            nc.vector.tensor_copy(bcs[:], bcp[:])
                        zv = zh_c[:].rearrange("p (q j m) -> p q j m",
                                               q=4, j=NP)
                        pzv0 = pz[0][0:64, :].rearrange("p (q m) -> p q m", q=4)
                        pzv1 = pz[1][0:64, :].rearrange("p (q m) -> p q m", q=4)
                        bv0 = bcs[0:64, :].rearrange("p (q m) -> p q m", q=4)
                        bv1 = bcs[64:128, :].rearrange("p (q m) -> p q m", q=4)
                        nc.vector.tensor_mul(zv[0:64, :, j, :], pzv0, bv0)
                        nc.vector.tensor_mul(zv[64:128, :, j, :], pzv1, bv1)
                        ew_book("dve", 1024 * 1.04 + 360)

                    deferred.append(norm_b)

                # chunk tail: W_O + out DMA, deferred into the next chunk
                def chunk_tail(c=c, zh_c=zh_c):
                    zv = zh_c[:].rearrange("p (q j m) -> p q j m", q=4, j=NP)
                    for qs in range(4):
                        for half in range(2):
                            po = mspool.tile([128, 512], f32, tag="ms",
                                             name="po")
                            hsl = slice(half * 384, (half + 1) * 384)
                            for j in range(NP):
                                nc.tensor.matmul(
                                    po[:, 0:384], zv[:, qs, j, :],
                                    wo_sb[:, j * 768 + half * 384:
                                          j * 768 + (half + 1) * 384],
                                    start=(j == 0), stop=(j == NP - 1))
                            osb = opool.tile([128, 384], f32, tag="osb",
                                             name="osb")
                            eng = ew_pick(384 * 0.833, 384 * 1.04)
                            if eng == "act":
                                nc.scalar.copy(osb[:], po[:, 0:384])
                            else:
                                nc.vector.tensor_copy(osb[:], po[:, 0:384])
                            row = c * 512 + qs * 128
                            q_e = nc.sync if (qs + half) % 2 == 0 else nc.gpsimd
                            q_e.dma_start(out[row:row + 128, hsl], osb[:])

                deferred.append(chunk_tail)

                for u in filler:
                    u()

            for th in deferred:
                th()

    _split_drain_waits(nc, mybir)
    return nc


_nc_cache = None


def _prep_core_inputs(x, W_Q, W_K, W_V, W_O, b_Q, b_K, core):
    import ml_dtypes
    bf16 = ml_dtypes.bfloat16
    b, g = core // G, core % G
    hs = slice(g * HPG, (g + 1) * HPG)

    xb = np.asarray(x[b], np.float32)               # [2048, 768]
    xT = np.ascontiguousarray(xb.T).astype(bf16)

    def wqk(W):
        # [p, a, (j hh d)] <- W[2j+hh, a*128+p, d]
        w = np.asarray(W, np.float32)[hs]           # [6, 768, 64]
        return np.ascontiguousarray(
            w.reshape(3, 2, 6, 128, 64).transpose(3, 2, 0, 1, 4)
            .reshape(128, -1)).astype(bf16)

    def wvp(W):
        # [p, a, (h d)] <- W[h, a*128+p, d]
        w = np.asarray(W, np.float32)[hs]
        return np.ascontiguousarray(
            w.reshape(6, 6, 128, 64).transpose(2, 1, 0, 3)
            .reshape(128, -1)).astype(bf16)

    # wo: [p=(hh*64+d), (j dcol)] <- W_O[2j+hh, d, dcol]
    wo = np.asarray(W_O, np.float32)[hs]            # [6, 64, 768]
    wod = np.ascontiguousarray(
        wo.reshape(3, 2, 64, 768).transpose(1, 2, 0, 3)
        .reshape(128, -1)).astype(bf16)

    def bp(bias):
        bb = np.asarray(bias, np.float32)[hs]       # [6, 64]
        return np.ascontiguousarray(
            bb.reshape(3, 2, 64).transpose(1, 2, 0).reshape(128, 3))

    p = np.arange(128)[:, None]
    q = np.arange(512)[None, :]
    m1 = np.where((q < 128) & (q < p), 0, M32).astype(np.int16)
    mA = np.concatenate([m1, m1], 1)

    sel2 = np.zeros((2, 128), bf16)
    sel2[0, 0:64] = 1
    sel2[1, 64:128] = 1

    return {
        "xT": xT, "wq": wqk(W_Q), "wk": wqk(W_K), "wv": wvp(W_V),
        "wo": wod, "bqd": bp(b_Q), "bkd": bp(b_K),
        "mskd": mA, "sel2d": sel2,
        "ones2_d": np.ones((128, 128), np.float32),
    }


def kernel(normalized_resid_pre, W_Q, W_K, W_V, W_O, b_Q, b_K, b_V, b_O):
    from concourse.bass_utils import run_bass_kernel_spmd

    global _nc_cache
    if _nc_cache is None:
        _nc_cache = build_program()
    nc = _nc_cache

    x = np.asarray(normalized_resid_pre, np.float32)
    in_maps = [_prep_core_inputs(x, W_Q, W_K, W_V, W_O, b_Q, b_K, c)
               for c in range(N_CORES)]

    res = run_bass_kernel_spmd(nc, in_maps, core_ids=list(range(N_CORES)))
    out = np.zeros((B, S, D), np.float32)
    for c in range(N_CORES):
        out[c // G] += np.asarray(res.results[c]["out"], np.float32)
    # bias folds: b_V rides through softmax rows (sum to 1) into W_O
    out += np.asarray(b_O, np.float32)
    out += np.einsum("nh,nhd->d", np.asarray(b_V, np.float32),
                     np.asarray(W_O, np.float32))
    return out
